# revision 1
# baseline (speedup 1.0000x reference)
"""CG-SENSE MRI reconstruction (nn_CGClass) on 8 Trainium2 NeuronCores.

Strategy: data-parallel over batch (B=8 -> 1 sample per core). Per core the
whole CG (10 iterations) runs on-chip. fft2/ifft2 are dense 320-point DFTs
done on the tensor engine as fp16 matmuls via the transpose-free primitive
OUT = Z^T @ A (data stationary, DFT matrix moving); applying it twice gives
F @ Z @ F with no transposes. CG state stays fp32; only matmul operands are
fp16 (measured end-to-end error ~3e-4, reference tol is far looser).

Layout: each 320x320 real array lives in SBUF as [128, 960]: free-dim block
t in {0,1,2} holds image rows [128t : 128t+{128,128,64}]. Block 2 uses
partitions 0..63; pad regions are kept zero (NaN hygiene for reductions).
"""
import os
from contextlib import ExitStack

import numpy as np

import concourse.bass as bass  # noqa: F401
import concourse.tile as tile
from concourse import mybir, bass_utils, bacc

F32 = mybir.dt.float32
F16 = mybir.dt.float16
MULT = mybir.AluOpType.mult
ADD = mybir.AluOpType.add

H = 320
B, C = 8, 12
N_ITER = int(os.environ.get("CG_ITERS", "10"))
KT = [(0, 128), (128, 128), (256, 64)]  # (row_start, rows) per block

_PROGRAM = None
TRACE = bool(os.environ.get("CG_TRACE"))
UNROLL = bool(os.environ.get("CG_UNROLL"))


def _mblk(t):
    return slice(320 * t, 320 * t + 320)


def _build_consts():
    j = np.arange(H)
    ang = -2.0 * np.pi * np.outer(j, j) / H
    scale = 1.0 / np.sqrt(H)
    Fr = (np.cos(ang) * scale).astype(np.float32)
    Fi = (np.sin(ang) * scale).astype(np.float32)

    def blocks(a):
        out = np.zeros((128, 960), np.float32)
        for t, (s, sz) in enumerate(KT):
            out[:sz, 320 * t:320 * t + 320] = a[s:s + sz]
        # block-2 rows duplicated at partitions 64..127 for row-group-packed
        # K=64 matmuls (stationary pairs at array rows 0-63 / 64-127)
        out[64:128, 640:960] = out[0:64, 640:960]
        return out

    return {
        "c_fr": blocks(Fr).astype(np.float16),
        "c_fi": blocks(Fi).astype(np.float16),
        "c_nfi": blocks(-Fi).astype(np.float16),
        "ones_col": np.ones((128, 1), np.float32),
        "ones_row": np.ones((1, 128), np.float32),
    }


def _build_program():
    nc = bacc.Bacc("TRN2", target_bir_lowering=False, debug=False)

    d = {}
    d["x_re"] = nc.dram_tensor("x_re", [H, H], F32, kind="ExternalInput")
    d["x_im"] = nc.dram_tensor("x_im", [H, H], F32, kind="ExternalInput")
    d["y_re"] = nc.dram_tensor("y_re", [C, H, H], F32, kind="ExternalInput")
    d["y_im"] = nc.dram_tensor("y_im", [C, H, H], F32, kind="ExternalInput")
    d["s_re"] = nc.dram_tensor("s_re", [C, H, H], F16, kind="ExternalInput")
    d["s_im"] = nc.dram_tensor("s_im", [C, H, H], F16, kind="ExternalInput")
    d["mask"] = nc.dram_tensor("mask", [H, H], F32, kind="ExternalInput")
    d["lam_b"] = nc.dram_tensor("lam_b", [128, 1], F32, kind="ExternalInput")
    d["c_fr"] = nc.dram_tensor("c_fr", [128, 960], F16, kind="ExternalInput")
    d["c_fi"] = nc.dram_tensor("c_fi", [128, 960], F16, kind="ExternalInput")
    d["c_nfi"] = nc.dram_tensor("c_nfi", [128, 960], F16, kind="ExternalInput")
    d["ones_col"] = nc.dram_tensor("ones_col", [128, 1], F32, kind="ExternalInput")
    d["ones_row"] = nc.dram_tensor("ones_row", [1, 128], F32, kind="ExternalInput")
    d["out"] = nc.dram_tensor("out", [2, H, H], F32, kind="ExternalOutput")

    with tile.TileContext(nc) as tc, ExitStack() as ctx:
        persist = ctx.enter_context(tc.tile_pool(name="persist", bufs=1))
        stg16 = ctx.enter_context(tc.tile_pool(name="stg16", bufs=2))
        tmp32 = ctx.enter_context(tc.tile_pool(name="tmp32", bufs=2))
        ps = ctx.enter_context(tc.tile_pool(name="ps", bufs=3, space="PSUM"))
        pss = ctx.enter_context(tc.tile_pool(name="pss", bufs=1, space="PSUM"))

        def load_blocks(dst, src_ap):
            for t, (s, sz) in enumerate(KT):
                nc.sync.dma_start(dst[0:sz, _mblk(t)], src_ap[s:s + sz, :])

        def zero_pad(t32, eng=None):
            (eng or nc.vector).memset(t32[64:128, 640:960], 0.0)

        # ---- persistent tiles ----
        sr = [persist.tile([128, 960], F16, tag=f"sr{c}", name=f"sr{c}") for c in range(C)]
        si = [persist.tile([128, 960], F16, tag=f"si{c}", name=f"si{c}") for c in range(C)]
        mask2 = persist.tile([128, 960], F32, tag="mask2", name="mask2")
        mask1 = persist.tile([128, 960], F32, tag="mask1", name="mask1")
        p_re = persist.tile([128, 960], F32, tag="p_re", name="p_re")
        p_im = persist.tile([128, 960], F32, tag="p_im", name="p_im")
        r_re = persist.tile([128, 960], F32, tag="r_re", name="r_re")
        r_im = persist.tile([128, 960], F32, tag="r_im", name="r_im")
        x_re = persist.tile([128, 960], F32, tag="x_re", name="x_re")
        x_im = persist.tile([128, 960], F32, tag="x_im", name="x_im")
        ap_re = persist.tile([128, 960], F32, tag="ap_re", name="ap_re")
        ap_im = persist.tile([128, 960], F32, tag="ap_im", name="ap_im")
        fr = persist.tile([128, 960], F16, tag="fr", name="fr")
        fi = persist.tile([128, 960], F16, tag="fi", name="fi")
        nfi = persist.tile([128, 960], F16, tag="nfi", name="nfi")
        ones_col = persist.tile([128, 1], F32, tag="ones_col", name="ones_col")
        ones_row = persist.tile([1, 128], F32, tag="ones_row", name="ones_row")
        lam_b = persist.tile([128, 1], F32, tag="lam_b", name="lam_b")
        alpha_b = persist.tile([128, 1], F32, tag="alpha_b", name="alpha_b")
        beta_b = persist.tile([128, 1], F32, tag="beta_b", name="beta_b")
        dacc = persist.tile([128, 2], F32, tag="dacc", name="dacc")
        sc = persist.tile([1, 8], F32, tag="sc", name="sc")
        # sc slots: 0=rTr, 1=inv_rTr, 2=pAp, 3=alpha, 4=rTrNew, 5=beta, 6=tmp
        scr = persist.tile([128, 960], F32, tag="scr", name="scr")
        scr2 = persist.tile([128, 960], F32, tag="scr2", name="scr2")
        jnk = persist.tile([128, 960], F32, tag="jnk", name="jnk")

        def emit_side(zr16, zi16, chain, consume):
            """psum(re,im) per m-block of Z^T @ A, complex. consume(mt,msz,pre,pim)."""
            if chain == "F":
                mov = [(zr16, fr, "re"), (zr16, fi, "im"),
                       (zi16, nfi, "re"), (zi16, fr, "im")]
            else:  # G = conj(F)
                mov = [(zr16, fr, "re"), (zr16, nfi, "im"),
                       (zi16, fi, "re"), (zi16, fr, "im")]
            for mt, (ms, msz) in enumerate(KT):
                pre = ps.tile([128, 320], F32, tag="ps_re", name="ps_re", bufs=4)
                pim = ps.tile([128, 320], F32, tag="ps_im", name="ps_im")
                cnt = {"re": 0, "im": 0}
                for kt, (ks, ksz) in enumerate(KT):
                    for zi_idx, (z, a, dst) in enumerate(mov):
                        lo = 320 * kt + 128 * mt
                        pt = (pre if dst == "re" else pim)[0:msz, :]
                        cnt[dst] += 1
                        if kt == 2 and z is zi16 and os.environ.get("CG_PACK"):
                            # packed partner: zi block-2 duplicated at
                            # partitions 64-127, concurrent with the zr MM
                            zt = z[64:128, lo:lo + msz]
                            at = a[64:128, _mblk(kt)]
                        else:
                            zt = z[0:ksz, lo:lo + msz]
                            at = a[0:ksz, _mblk(kt)]
                        nc.tensor.matmul(pt, zt, at, start=(cnt[dst] == 1),
                                         stop=(cnt[dst] == 6))
                consume(mt, msz, pre, pim)

        def cmul_to_fp16(ar, ai, br, bi, outr, outi):
            """(outr + i outi) = (ar + i ai)(br + i bi); fp32 in, fp16 out."""
            t1 = tmp32.tile([128, 960], F32, tag="mm_t1", name="mm_t1")
            t2 = tmp32.tile([128, 960], F32, tag="mm_t2", name="mm_t2")
            t3 = tmp32.tile([128, 960], F32, tag="mm_t3", name="mm_t3")
            t4 = tmp32.tile([128, 960], F32, tag="mm_t4", name="mm_t4")
            nc.gpsimd.tensor_mul(t1[:], ar[:], br[:])
            nc.gpsimd.tensor_mul(t2[:], ai[:], bi[:])
            nc.gpsimd.tensor_mul(t3[:], ar[:], bi[:])
            nc.gpsimd.tensor_mul(t4[:], ai[:], br[:])
            nc.vector.tensor_sub(outr[:], t1[:], t2[:])
            nc.vector.tensor_add(outi[:], t3[:], t4[:])
            nc.sync.dma_start(outi[64:128, 640:960], outi[0:64, 640:960])

        def combine_coil(c, u4r, u4i):
            """ap += conj(s_c) * u4 (fp32)."""
            t1 = tmp32.tile([128, 960], F32, tag="mm_t1", name="mm_t1")
            t2 = tmp32.tile([128, 960], F32, tag="mm_t2", name="mm_t2")
            t3 = tmp32.tile([128, 960], F32, tag="mm_t3", name="mm_t3")
            t4 = tmp32.tile([128, 960], F32, tag="mm_t4", name="mm_t4")
            nc.vector.tensor_mul(t1[:], sr[c][:], u4r[:])
            nc.vector.tensor_mul(t2[:], si[c][:], u4i[:])
            nc.vector.tensor_mul(t3[:], sr[c][:], u4i[:])
            nc.vector.tensor_mul(t4[:], si[c][:], u4r[:])
            nc.vector.tensor_add(ap_re[:], ap_re[:], t1[:])
            nc.vector.tensor_add(ap_re[:], ap_re[:], t2[:])
            nc.vector.tensor_add(ap_im[:], ap_im[:], t3[:])
            nc.vector.tensor_sub(ap_im[:], ap_im[:], t4[:])

        def ifft_and_combine(c, inr16, ini16):
            """Emit S3/S4 G-chain; return deferred combine closure."""
            s3r = stg16.tile([128, 960], F16, tag="s3r", name="s3r")
            s3i = stg16.tile([128, 960], F16, tag="s3i", name="s3i")

            def consume3(mt, msz, pre, pim):
                nc.scalar.copy(s3r[0:msz, _mblk(mt)], pre[0:msz, :])
                nc.scalar.copy(s3i[0:msz, _mblk(mt)], pim[0:msz, :])
                if mt == 2:
                    nc.sync.dma_start(s3i[64:128, 640:960], s3i[0:64, 640:960])
            emit_side(inr16, ini16, "G", consume3)

            u4r = tmp32.tile([128, 960], F32, tag="u4r", name="u4r")
            u4i = tmp32.tile([128, 960], F32, tag="u4i", name="u4i")
            zero_pad(u4r, nc.gpsimd)
            zero_pad(u4i, nc.gpsimd)

            def consume4(mt, msz, pre, pim):
                nc.scalar.copy(u4r[0:msz, _mblk(mt)], pre[0:msz, :])
                nc.scalar.copy(u4i[0:msz, _mblk(mt)], pim[0:msz, :])
            emit_side(s3r, s3i, "G", consume4)
            return lambda: combine_coil(c, u4r, u4i)

        def dot_to_sc(a_re, b_re, a_im, b_im, slot):
            """sc[0, slot] = sum(a_re*b_re + a_im*b_im) over valid region.

            tensor_tensor_reduce faults on this hardware path, so: self-dots
            use ACT Square+accum_out; cross-dots DVE-mult + ACT Copy+accum.
            """
            SQ = mybir.ActivationFunctionType.Square
            CP = mybir.ActivationFunctionType.Copy
            if a_re is b_re and a_im is b_im:
                nc.scalar.activation(jnk[:], a_re[:], SQ, accum_out=dacc[:, 0:1])
                nc.scalar.activation(jnk[:], a_im[:], SQ, accum_out=dacc[:, 1:2])
            else:
                nc.vector.tensor_mul(scr[:], a_re[:], b_re[:])
                nc.vector.tensor_mul(scr2[:], a_im[:], b_im[:])
                nc.scalar.activation(jnk[:], scr[:], CP, accum_out=dacc[:, 0:1])
                nc.scalar.activation(jnk[:], scr2[:], CP, accum_out=dacc[:, 1:2])
            pd = pss.tile([1, 2], F32, tag="pdot", name="pdot")
            nc.tensor.matmul(pd[0:1, 0:2], ones_col[:, 0:1], dacc[:, 0:2],
                             start=True, stop=True)
            nc.vector.tensor_copy(sc[0:1, 6:8], pd[0:1, 0:2])
            nc.vector.tensor_add(sc[0:1, slot:slot + 1], sc[0:1, 6:7],
                                 sc[0:1, 7:8])

        # ---- load constants + inputs ----
        nc.sync.dma_start(fr[:], d["c_fr"].ap())
        nc.sync.dma_start(fi[:], d["c_fi"].ap())
        nc.sync.dma_start(nfi[:], d["c_nfi"].ap())
        nc.sync.dma_start(ones_col[:], d["ones_col"].ap())
        nc.sync.dma_start(ones_row[:], d["ones_row"].ap())
        nc.sync.dma_start(lam_b[:], d["lam_b"].ap())
        for c in range(C):
            load_blocks(sr[c], d["s_re"].ap()[c])
            load_blocks(si[c], d["s_im"].ap()[c])
            zero_pad(sr[c])
            zero_pad(si[c])
        load_blocks(mask1, d["mask"].ap())
        zero_pad(mask1)
        load_blocks(x_re, d["x_re"].ap())
        load_blocks(x_im, d["x_im"].ap())
        zero_pad(x_re)
        zero_pad(x_im)
        nc.vector.tensor_mul(mask2[:], mask1[:], mask1[:])

        nc.vector.memset(ap_re[:], 0.0)
        nc.vector.memset(ap_im[:], 0.0)

        # ---- phase 1: rhs ----
        _dbg = os.environ.get("CG_DEBUG", "")
        _ncoil = 0 if _dbg == "loadonly" else (1 if _dbg == "coil1" else C)
        def make_my(c):
            yr = tmp32.tile([128, 960], F32, tag="yr", name="yr")
            yi = tmp32.tile([128, 960], F32, tag="yi", name="yi")
            load_blocks(yr, d["y_re"].ap()[c])
            load_blocks(yi, d["y_im"].ap()[c])
            zero_pad(yr, nc.gpsimd)
            zero_pad(yi, nc.gpsimd)
            myr = stg16.tile([128, 960], F16, tag="spr", name="myr")
            myi = stg16.tile([128, 960], F16, tag="spi", name="myi")
            nc.vector.tensor_mul(myr[:], yr[:], mask1[:])
            nc.vector.tensor_mul(myi[:], yi[:], mask1[:])
            nc.sync.dma_start(myi[64:128, 640:960], myi[0:64, 640:960])
            return myr, myi

        my_next = make_my(0) if _ncoil else None
        pending = None
        for c in range(_ncoil):
            myr, myi = my_next
            if c + 1 < _ncoil:
                my_next = make_my(c + 1)
            comb = ifft_and_combine(c, myr, myi)
            if pending is not None:
                pending()
            pending = comb
        if pending is not None:
            pending()

        # r0 = p0 = rhs = ap + lam*x ; x0 = 0
        nc.vector.tensor_scalar_mul(scr[:], x_re[:], lam_b[:, 0:1])
        nc.vector.tensor_add(r_re[:], ap_re[:], scr[:])
        nc.vector.tensor_scalar_mul(scr2[:], x_im[:], lam_b[:, 0:1])
        nc.vector.tensor_add(r_im[:], ap_im[:], scr2[:])
        nc.scalar.copy(p_re[:], r_re[:])
        nc.scalar.copy(p_im[:], r_im[:])
        nc.vector.memset(x_re[:], 0.0)
        nc.vector.memset(x_im[:], 0.0)

        dot_to_sc(r_re, r_re, r_im, r_im, 0)          # rTr0
        nc.vector.reciprocal(sc[0:1, 1:2], sc[0:1, 0:1])

        # ---- phase 2: CG iterations ----
        def cg_iteration():
            nc.vector.tensor_scalar_mul(ap_re[:], p_re[:], lam_b[:, 0:1])
            nc.vector.tensor_scalar_mul(ap_im[:], p_im[:], lam_b[:, 0:1])

            def make_sp(c):
                spr = stg16.tile([128, 960], F16, tag="spr", name="spr")
                spi = stg16.tile([128, 960], F16, tag="spi", name="spi")
                cmul_to_fp16(sr[c], si[c], p_re, p_im, spr, spi)
                return spr, spi

            sp_next = make_sp(0)
            pending = None
            for c in range(C):
                spr, spi = sp_next
                s1r = stg16.tile([128, 960], F16, tag="s1r", name="s1r")
                s1i = stg16.tile([128, 960], F16, tag="s1i", name="s1i")

                def consume1(mt, msz, pre, pim):
                    nc.scalar.copy(s1r[0:msz, _mblk(mt)], pre[0:msz, :])
                    nc.scalar.copy(s1i[0:msz, _mblk(mt)], pim[0:msz, :])
                    if mt == 2:
                        nc.sync.dma_start(s1i[64:128, 640:960],
                                          s1i[0:64, 640:960])
                emit_side(spr, spi, "F", consume1)

                wr = stg16.tile([128, 960], F16, tag="wr", name="wr")
                wi = stg16.tile([128, 960], F16, tag="wi", name="wi")

                def consume2(mt, msz, pre, pim):
                    nc.vector.tensor_mul(wr[0:msz, _mblk(mt)], pre[0:msz, :],
                                         mask2[0:msz, _mblk(mt)])
                    nc.vector.tensor_mul(wi[0:msz, _mblk(mt)], pim[0:msz, :],
                                         mask2[0:msz, _mblk(mt)])
                    if mt == 2:
                        nc.sync.dma_start(wi[64:128, 640:960],
                                          wi[0:64, 640:960])
                emit_side(s1r, s1i, "F", consume2)
                if pending is not None:
                    pending()

                # prepare next coil's SP before this coil's ifft+combine so the
                # DVE/GpSimd streams feed the PE ahead of the combine chain
                if c + 1 < C:
                    sp_next = make_sp(c + 1)
                pending = ifft_and_combine(c, wr, wi)
            pending()

            dot_to_sc(p_re, ap_re, p_im, ap_im, 2)    # pAp
            nc.vector.reciprocal(sc[0:1, 6:7], sc[0:1, 2:3])
            nc.vector.tensor_mul(sc[0:1, 3:4], sc[0:1, 0:1], sc[0:1, 6:7])  # alpha
            if os.environ.get("CG_NOBCASTMM"):
                nc.gpsimd.partition_broadcast(alpha_b[:, 0:1], sc[0:1, 3:4])
            else:
                pb = pss.tile([128, 1], F32, tag="pdot", name="pbc")
                nc.tensor.matmul(pb[:, 0:1], ones_row[0:1, :], sc[0:1, 3:4],
                                 start=True, stop=True)
                nc.scalar.copy(alpha_b[:, 0:1], pb[:, 0:1])

            nc.vector.tensor_scalar_mul(scr[:], ap_re[:], alpha_b[:, 0:1])
            nc.vector.tensor_sub(r_re[:], r_re[:], scr[:])
            nc.vector.tensor_scalar_mul(scr2[:], ap_im[:], alpha_b[:, 0:1])
            nc.vector.tensor_sub(r_im[:], r_im[:], scr2[:])

            dot_to_sc(r_re, r_re, r_im, r_im, 4)      # rTrNew
            nc.vector.tensor_mul(sc[0:1, 5:6], sc[0:1, 4:5], sc[0:1, 1:2])  # beta
            nc.vector.tensor_copy(sc[0:1, 0:1], sc[0:1, 4:5])
            nc.vector.reciprocal(sc[0:1, 1:2], sc[0:1, 4:5])
            if os.environ.get("CG_NOBCASTMM"):
                nc.gpsimd.partition_broadcast(beta_b[:, 0:1], sc[0:1, 5:6])
            else:
                pb2 = pss.tile([128, 1], F32, tag="pdot", name="pbc2")
                nc.tensor.matmul(pb2[:, 0:1], ones_row[0:1, :], sc[0:1, 5:6],
                                 start=True, stop=True)
                nc.scalar.copy(beta_b[:, 0:1], pb2[:, 0:1])

            nc.scalar.mul(scr[:], p_re[:], beta_b[:, 0:1])
            nc.scalar.mul(scr2[:], p_im[:], beta_b[:, 0:1])
            nc.vector.tensor_scalar(jnk[:], p_re[:], alpha_b[:, 0:1], None, MULT)
            nc.vector.tensor_add(x_re[:], x_re[:], jnk[:])
            nc.vector.tensor_add(p_re[:], r_re[:], scr[:])
            nc.vector.tensor_scalar(scr[:], p_im[:], alpha_b[:, 0:1], None, MULT)
            nc.vector.tensor_add(p_im[:], r_im[:], scr2[:])
            nc.vector.tensor_add(x_im[:], x_im[:], scr[:])

        dbg = _dbg
        if dbg in ("rhs", "loadonly", "coil1"):
            nc.scalar.copy(x_re[:], r_re[:])
            nc.scalar.copy(x_im[:], r_im[:])
        elif dbg == "setup":
            nc.scalar.copy(x_re[:], mask2[:])
            nc.scalar.copy(x_im[:], mask2[:])
        elif UNROLL:
            for _ in range(N_ITER):
                cg_iteration()
        else:
            with tc.For_i(0, N_ITER, 1):
                cg_iteration()

        for t, (s, sz) in enumerate(KT):
            nc.sync.dma_start(d["out"].ap()[0, s:s + sz, :], x_re[0:sz, _mblk(t)])
            nc.sync.dma_start(d["out"].ap()[1, s:s + sz, :], x_im[0:sz, _mblk(t)])

    nc.compile()
    return nc


def kernel(lambdaa, x_re, x_im, y_re, y_im, smaps_re, smaps_im, mask):
    global _PROGRAM
    lambdaa = np.asarray(lambdaa, np.float32)
    arrs = {
        "x_re": x_re, "x_im": x_im, "y_re": y_re, "y_im": y_im,
    }
    arrs = {k: np.ascontiguousarray(np.asarray(v, np.float32))
            for k, v in arrs.items()}
    arrs["s_re"] = np.ascontiguousarray(np.asarray(smaps_re, np.float16))
    arrs["s_im"] = np.ascontiguousarray(np.asarray(smaps_im, np.float16))
    mask = np.ascontiguousarray(np.asarray(mask, np.float32))

    if _PROGRAM is None:
        _PROGRAM = _build_program()
    nc = _PROGRAM

    consts = _build_consts()
    lam_b = np.full((128, 1), float(lambdaa[0]), np.float32)
    in_maps = []
    for i in range(B):
        in_maps.append({
            **{k: v[i] for k, v in arrs.items()},
            "mask": np.ascontiguousarray(mask[i, 0]),
            "lam_b": lam_b,
            **consts,
        })

    res = bass_utils.run_bass_kernel_spmd(nc, in_maps, core_ids=list(range(B)),
                                          trace=TRACE)
    kernel._last_result = res
    out = np.empty((B, H, H, 2), np.float32)
    for i in range(B):
        o = res.results[i]["out"]
        out[i, :, :, 0] = o[0]
        out[i, :, :, 1] = o[1]
    return out

